# revision 1
# baseline (speedup 1.0000x reference)
"""Trainium2 Bass kernel for the 2-layer custom LSTM model.

Reference semantics (single (h, c) pair shared across both layers):
    x_t = emb[texts[t]]                           # [B, E]
    layer 0: cat = [h; x_t]   (K = H + E = 1024)
    layer 1: cat = [h'; h']   (so W1_eff = W1[:, :H] + W1[:, H:], K = 512)
    gates: f,i,o = sigmoid(W cat), chat = tanh(W cat); c = f*c + i*chat;
    h = o * tanh(c)
    y = h_final^T @ Wy^T + by^T                   # [B, OUT]

Strategy: all 8 cores run an identical replicated program (weights
replicated; recurrence is sequential in time so there is no useful way to
split the tiny per-step GEMMs without paying per-step cross-core latency
that exceeds the compute itself).  The input projections W_x @ x_t for all
timesteps are hoisted out of the recurrence and computed as one big GEMM
(phase B); only the recurrent half W_h @ h stays on the sequential path.

Layouts:
  - gate-column order: col = k*512 + g*128 + j  (k = hidden chunk 0..3,
    g = gate f,i,c,o, j = 0..127), so each 512-wide PSUM chunk holds all 4
    gates for one 128-wide hidden chunk.
  - recurrence matmuls are "h-stationary": G^T[64b, 2048] = h^T @ W^T with
    lhsT = h [K=512 hidden, M=64 batch], rhs = W^T [512, 2048] moving.
  - h^T chunks [64, 128] are transposed back to [128, 64] k-tiles on the PE
    for the next step's lhsT.
"""

import os
from contextlib import ExitStack

import numpy as np

import concourse.bass as bass
import concourse.mybir as mybir
import concourse.tile as tile
from concourse import bacc
from concourse.bass import ds, ts
from concourse.bass_utils import run_bass_kernel_spmd
from concourse.masks import make_identity

AF = mybir.ActivationFunctionType
F32 = mybir.dt.float32
F32R = mybir.dt.float32r
I32 = mybir.dt.int32

V, E, H, OUT, L = 32000, 512, 512, 2, 2
S, B = 512, 64
G4 = 4 * H  # 2048 stacked gate dim
NK = H // 128  # 4 k-tiles of hidden
NCHUNK = 4  # 512-wide gate chunks per layer


def _interleave_cols(w):  # w [2048(g-major), K] -> [K, 2048] (cols g-major)
    return np.ascontiguousarray(w.T)


def build_nc(n_steps=S, unroll=8, prep_unroll=4, repeat=1, skip_prep=False,
             static_loops=False, pre_tiles=8, hint=False, stag=False):
    nc = bacc.Bacc("TRN2", target_bir_lowering=False, debug=False, num_devices=8)

    texts_d = nc.dram_tensor("texts", [S * B, 1], I32, kind="ExternalInput").ap()
    emb_d = nc.dram_tensor("emb", [V, E], F32, kind="ExternalInput").ap()
    wx0_d = nc.dram_tensor("wx0T", [E, G4], F32R, kind="ExternalInput").ap()
    wh0_d = nc.dram_tensor("wh0T", [H, G4], F32R, kind="ExternalInput").ap()
    wh1_d = nc.dram_tensor("wh1T", [H, G4], F32R, kind="ExternalInput").ap()
    wy_d = nc.dram_tensor("wyT", [H, OUT], F32R, kind="ExternalInput").ap()
    y_d = nc.dram_tensor("y", [B, OUT], F32, kind="ExternalOutput").ap()

    x4_d = nc.dram_tensor("x4T", [S * B, G4], F32R).ap()  # internal scratch

    n_tiles = (n_steps * B) // 128
    pre = min(pre_tiles, n_tiles)

    with tile.TileContext(nc) as tc, ExitStack() as ctx:
        # ---------- constants / persistent weights ----------
        consts = ctx.enter_context(tc.tile_pool(name="consts", bufs=1))
        ident128 = consts.tile([128, 128], F32)
        make_identity(nc, ident128[:])
        ident64 = consts.tile([64, 64], F32)
        make_identity(nc, ident64[:])
        identr = consts.tile([64, 64], F32R)
        nc.vector.tensor_copy(identr[:], ident64[:])

        wpool = ctx.enter_context(tc.tile_pool(name="weights", bufs=1))
        wh0_sb = [wpool.tile([128, G4], F32R, tag=f"wh0_{q}", name=f"wh0_{q}")
                  for q in range(NK)]
        wh1_sb = [wpool.tile([128, G4], F32R, tag=f"wh1_{q}", name=f"wh1_{q}")
                  for q in range(NK)]
        wx0_sb = [wpool.tile([128, G4], F32R, tag=f"wx0_{q}", name=f"wx0_{q}")
                  for q in range(NK)]
        wy_sb = wpool.tile([128, NK * OUT], F32R)
        for q in range(NK):
            nc.sync.dma_start(wh0_sb[q][:], wh0_d[ts(q, 128), :])
            nc.sync.dma_start(wh1_sb[q][:], wh1_d[ts(q, 128), :])
            nc.sync.dma_start(wx0_sb[q][:], wx0_d[ts(q, 128), :])
            nc.sync.dma_start(wy_sb[:, ts(q, OUT)], wy_d[ts(q, 128), :])

        # ---------- state ----------
        state = ctx.enter_context(tc.tile_pool(name="state", bufs=1))
        h_sb = state.tile([128, NK * B], F32R, tag="h")   # k-tile q at cols 64q
        h2_sb = state.tile([128, NK * B], F32R, tag="h2")
        c_sb = state.tile([64, H], F32, tag="c")
        c2_sb = state.tile([64, H], F32, tag="c2")
        nc.vector.memset(h_sb[:].bitcast(F32), 0.0)
        nc.vector.memset(c_sb[:], 0.0)

        # ---------- pools ----------
        inner = ctx.enter_context(ExitStack())
        pbs = inner.enter_context(tc.tile_pool(name="pb_sb", bufs=4))
        pbx = inner.enter_context(tc.tile_pool(name="pb_xt", bufs=3))
        pbo = inner.enter_context(tc.tile_pool(name="pb_out", bufs=2))
        pbp = inner.enter_context(tc.tile_pool(name="pb_ps", bufs=2, space="PSUM"))
        pbt = inner.enter_context(tc.tile_pool(name="pb_tp", bufs=1, space="PSUM"))
        pcx = inner.enter_context(tc.tile_pool(name="pc_x4", bufs=3))
        pct = inner.enter_context(tc.tile_pool(name="pc_tmp", bufs=3))
        pch = inner.enter_context(tc.tile_pool(name="pc_ht", bufs=2))
        pcp = inner.enter_context(tc.tile_pool(name="pc_ps", bufs=1, space="PSUM"))
        pctp = inner.enter_context(tc.tile_pool(name="pc_tp", bufs=1, space="PSUM"))

        # ---------- phase B body: gather + input projection for one tile ----
        def prep_body(t):
            idx = pbs.tile([128, 1], I32, tag="idx")
            nc.sync.dma_start(idx[:], texts_d[ds(t * 128, 128), :])
            gx = pbs.tile([128, E], F32, tag="gx")
            nc.gpsimd.indirect_dma_start(
                out=gx[:],
                out_offset=None,
                in_=emb_d[:],
                in_offset=bass.IndirectOffsetOnAxis(ap=idx[:, :1], axis=0),
            )
            xt = [pbx.tile([128, 128], F32R, tag=f"xt{q}", name=f"xt{q}")
                  for q in range(NK)]
            for q in range(NK):
                tp = pbt.tile([128, 128], F32, name="pb_tp_t")
                nc.tensor.transpose(tp[:], gx[:, ts(q, 128)], ident128[:])
                if q % 2 == 0:
                    nc.vector.tensor_copy(xt[q][:], tp[:])
                else:
                    nc.scalar.copy(xt[q][:], tp[:])
            x4o = pbo.tile([128, G4], F32R, tag="x4o")
            for n in range(4):
                ps = pbp.tile([128, 512], F32, name="x4ps")
                for q in range(NK):
                    nc.tensor.matmul(
                        ps[:],
                        lhsT=xt[q][:],
                        rhs=wx0_sb[q][:, ts(n, 512)],
                        start=(q == 0),
                        stop=(q == NK - 1),
                    )
                if n % 2 == 0:
                    nc.vector.tensor_copy(x4o[:, ts(n, 512)], ps[:])
                else:
                    nc.scalar.copy(x4o[:, ts(n, 512)], ps[:])
            nc.sync.dma_start(x4_d[ds(t * 128, 128), :], x4o[:])

        # ---------- recurrence cell ----------
        # gate-major layout: cols [F | I | C | O], within-gate plain hidden.
        def cell(h_in, h_out, w_sb, c_in, c_out, x4sb):
            Gs = [pcp.tile([64, 512], F32, name=f"G{g}", tag=f"G{g}")
                  for g in range(4)]

            def mm(g, q):
                nc.tensor.matmul(
                    Gs[g][:],
                    lhsT=h_in[:, ts(q, B)],
                    rhs=w_sb[q][:, ts(g, 512)],
                    start=(q == 0 and x4sb is None), stop=(q == 3),
                )

            def mmx(g):
                if x4sb is not None:
                    nc.tensor.matmul(
                        Gs[g][:],
                        lhsT=identr[:],
                        rhs=x4sb[:, ts(g, 512)],
                        start=True, stop=False,
                    )

            sg = pct.tile([64, G4], F32, tag="sg")

            def act(g):
                af = AF.Tanh if g == 2 else AF.Sigmoid
                nc.scalar.activation(sg[:, ts(g, 512)], Gs[g][:], af)

            # gate completion order F, C, I, O (chain needs F/C/I early, O
            # feeds only the final h-mul).  k2/k3 of each gate issue late
            # enough that the producer's hidden half B has landed.
            mmx(0); mm(0, 0); mm(0, 1)          # F: x4,k0,k1
            mmx(2); mm(2, 0); mm(2, 1)          # C: x4,k0,k1
            mm(0, 2); mm(0, 3); act(0)          # F done -> sigF
            mm(2, 2); mm(2, 3); act(2)          # C done -> tanhC
            mmx(1); mm(1, 0); mm(1, 1)          # I: x4,k0,k1
            mm(1, 2); mm(1, 3); act(1)          # I done -> sigI
            mmx(3); mm(3, 0); mm(3, 1)          # O: x4,k0,k1
            mm(3, 2); mm(3, 3); act(3)          # O done -> sigO

            hT = pch.tile([64, H], F32, tag="hT")
            tps = pctp.tile([128, 2 * 128], F32, name="tps")
            t1 = [pct.tile([64, 256], F32, tag=f"t1_{u}", name=f"t1_{u}")
                  for u in range(2)]
            t2 = [pct.tile([64, 256], F32, tag=f"t2_{u}", name=f"t2_{u}")
                  for u in range(2)]
            tct = [pct.tile([64, 256], F32, tag=f"tct_{u}", name=f"tct_{u}")
                   for u in range(2)]
            hs = [slice(256 * u, 256 * u + 256) for u in range(2)]

            def F_(u): return sg[:, 0 + 256 * u:256 + 256 * u]
            def I_(u): return sg[:, 512 + 256 * u:768 + 256 * u]
            def C_(u): return sg[:, 1024 + 256 * u:1280 + 256 * u]
            def O_(u): return sg[:, 1536 + 256 * u:1792 + 256 * u]

            # half A critical chain first; half B trails on DVE
            # half-A critical chain strictly first on DVE; half B trails
            nc.vector.tensor_mul(t1[0][:], F_(0), c_in[:, hs[0]])
            nc.vector.tensor_mul(t1[1][:], F_(1), c_in[:, hs[1]])
            nc.vector.tensor_mul(t2[0][:], I_(0), C_(0))
            nc.vector.tensor_add(c_out[:, hs[0]], t1[0][:], t2[0][:])
            nc.scalar.activation(tct[0][:], c_out[:, hs[0]], AF.Tanh)
            nc.vector.tensor_mul(hT[:, hs[0]], O_(0), tct[0][:])
            nc.tensor.transpose(tps[:, ts(0, B)], hT[:, ts(0, 128)], ident64[:])
            nc.tensor.transpose(tps[:, ts(1, B)], hT[:, ts(1, 128)], ident64[:])
            nc.scalar.copy(h_out[:, 0:128], tps[:, 0:128])
            with tc.high_priority(offset=-40):
                nc.vector.tensor_mul(t2[1][:], I_(1), C_(1))
                nc.vector.tensor_add(c_out[:, hs[1]], t1[1][:], t2[1][:])
                nc.scalar.activation(tct[1][:], c_out[:, hs[1]], AF.Tanh)
                nc.vector.tensor_mul(hT[:, hs[1]], O_(1), tct[1][:])
            nc.tensor.transpose(tps[:, ts(2, B)], hT[:, ts(2, 128)], ident64[:])
            nc.tensor.transpose(tps[:, ts(3, B)], hT[:, ts(3, 128)], ident64[:])
            nc.vector.tensor_copy(h_out[:, 128:256], tps[:, 128:256])
        def step_body(s):
            x4sb = pcx.tile([64, G4], F32R, tag="x4sb")
            nc.sync.dma_start(x4sb[:], x4_d[ds(s * B, B), :])
            cell(h_sb, h2_sb, wh0_sb, c_sb, c2_sb, x4sb)
            cell(h2_sb, h_sb, wh1_sb, c2_sb, c_sb, None)

        def stepprep_body(s):
            step_body(s)
            if s % 2 == 0:
                prep_body(s // 2 + pre)

        # ---------- main loops ----------
        # spread prep at one tile per TWO steps so the PE load stays under
        # the cell-chain period throughout (tile pre+s/2 is consumed at step
        # 2*(pre+s/2), always >= pre steps ahead)
        mid = min(2 * (n_tiles - pre), n_steps)
        if skip_prep:
            loops = [(0, n_steps, step_body, unroll)]
        else:
            loops = [(0, pre, prep_body, prep_unroll),
                     (0, mid, stepprep_body, unroll),
                     (mid, n_steps, step_body, unroll)]
        hint_engines = tuple(nc.engines.keys()) if hint else ()

        def run_loop(lo, hi, body, ur):
            n = hi - lo
            ur = min(ur, n)
            main = n - n % ur
            if main:
                with tc.For_i(lo, lo + main, ur, hint_engines=hint_engines,
                              staggered_reset=stag) as i:
                    for j in range(ur):
                        body(i + j)
            for r in range(lo + main, hi):
                body(r)

        for (lo, hi, body, ur) in loops:
            if hi <= lo:
                continue
            if static_loops:
                for i in range(lo, hi):
                    body(i)
            elif repeat > 1 and body is step_body:
                with tc.For_i(0, repeat, 1):
                    run_loop(lo, hi, body, ur)
            else:
                run_loop(lo, hi, body, ur)

        inner.close()

        # ---------- phase D: output projection ----------
        with tc.tile_pool(name="pd", bufs=1) as pd, \
             tc.tile_pool(name="pd_ps", bufs=1, space="PSUM") as pdp:
            yps = pdp.tile([64, OUT], F32)
            for q in range(NK):
                nc.tensor.matmul(
                    yps[:],
                    lhsT=h_sb[:, ts(q, B)],
                    rhs=wy_sb[:, ts(q, OUT)],
                    start=(q == 0),
                    stop=(q == NK - 1),
                )
            ysb = pd.tile([64, OUT], F32)
            nc.vector.tensor_copy(ysb[:], yps[:])
            nc.sync.dma_start(y_d[:], ysb[:])

    nc.compile()
    return nc


def prep_inputs(texts, emb, Wf, bf, Wi, bi, Wo, bo, Wc, bc, Wy, by):
    """Host-side layout prep. All heavy compute stays on device."""
    texts = np.asarray(texts)
    sl, bb = texts.shape
    texts_i = np.ascontiguousarray(texts.reshape(sl * bb, 1).astype(np.int32))
    # stacked gate weights, g order (f, i, c, o) to match kernel layout
    w0 = np.concatenate([np.asarray(w)[0] for w in (Wf, Wi, Wc, Wo)], axis=0)
    w1 = np.concatenate([np.asarray(w)[1] for w in (Wf, Wi, Wc, Wo)], axis=0)
    wh0T = _interleave_cols(w0[:, :H])  # [512, 2048]
    wx0T = _interleave_cols(w0[:, H:])  # [512, 2048]
    wh1T = _interleave_cols(w1[:, :H] + w1[:, H:])  # [512, 2048]
    wyT = np.ascontiguousarray(np.asarray(Wy).T.astype(np.float32))  # [512, 2]
    return {
        "texts": texts_i,
        "emb": np.ascontiguousarray(np.asarray(emb), dtype=np.float32),
        "wx0T": wx0T.astype(np.float32),
        "wh0T": wh0T.astype(np.float32),
        "wh1T": wh1T.astype(np.float32),
        "wyT": wyT,
    }


_NC_CACHE = {}


def kernel(**inputs) -> np.ndarray:
    # Fully static (unrolled) schedule: compiles in ~20s and runs ~35% faster
    # than the dynamic-loop variant (no back-edge barriers, global overlap).
    key = "static"
    if key not in _NC_CACHE:
        _NC_CACHE[key] = build_nc(n_steps=S, static_loops=True)
    nc = _NC_CACHE[key]
    in_map = prep_inputs(**inputs)
    res = run_bass_kernel_spmd(nc, [in_map] * 8, core_ids=list(range(8)))
    return res.results[0]["y"]





# revision 48
# speedup vs baseline: 1.4575x; 1.4575x over previous
"""Trainium2 Bass kernel for the 2-layer custom LSTM model.

Reference semantics (single (h, c) pair shared across both layers):
    x_t = emb[texts[t]]                           # [B, E]
    layer 0: cat = [h; x_t]   (K = H + E = 1024)
    layer 1: cat = [h'; h']   (so W1_eff = W1[:, :H] + W1[:, H:], K = 512)
    gates: f,i,o = sigmoid(W cat), chat = tanh(W cat); c = f*c + i*chat;
    h = o * tanh(c)
    y = h_final^T @ Wy^T + by^T                   # [B, OUT]

Strategy: all 8 cores run an identical replicated program (recurrence is
latency-bound; cross-core collectives per step would cost more than they
save).  The input projections W_x @ x_t are hoisted out of the recurrence
into phase B (one big GEMM per 128-token tile, interleaved with the
recurrence as PE stall-filler); only W_h @ h stays on the sequential path.

v2 layout ("column-pair tiling + hidden fold", all matmuls bf16):
  - Each gate matmul has M=64 (batch) so it only uses half the 128-wide PE
    array.  We run TWO matmuls concurrently in the two 64-column halves
    (tile_position (0,0) / (0,64)), roughly doubling PE throughput.
  - Gate-column order of W^T[512, 2048]:
      [F(0:256) | C(0:256) | F(256:512) | C(256:512) |
       I(0:256) | O(0:256) | I(256:512) | O(256:512)]
    PSUM tile FC[128, 512]: partitions 0:64 = batch x [F|C](hid 0:256)
    (array half 0), partitions 64:128 = batch x [F|C](hid 256:512) (half 1).
    So every gate lands "hidden-folded" as [128, 256]: all elementwise work
    (sigmoid/tanh, c update) runs on the full 128 partitions.
  - x4 (input projection) is added into PSUM by an identity matmul whose
    stationary is [128, 64] = [I64; 0] so it shares the 128x64 tiling mode
    with the gate matmuls (rhs bottom partitions are zeroed once).
  - c state is fp32 folded [128, 256]; h is transposed back to k-tile
    layout [128(hid), 4*64(batch)] in bf16 for the next step's lhsT.
"""

import os
from contextlib import ExitStack

import numpy as np

import concourse.bass as bass
import concourse.mybir as mybir
import concourse.tile as tile
from concourse import bacc
from concourse.bass import ds, ts
from concourse.bass_utils import run_bass_kernel_spmd
from concourse.masks import make_identity

AF = mybir.ActivationFunctionType
F32 = mybir.dt.float32
BF = mybir.dt.bfloat16
I32 = mybir.dt.int32

V, E, H, OUT, L = 32000, 512, 512, 2, 2
S, B = 512, 64
G4 = 4 * H  # 2048 stacked gate dim
NK = H // 128  # 4 k-tiles of hidden
HH = 256  # folded hidden per array half
XRING = 7  # x4 SBUF ring depth (bounds prep run-ahead to ~2*(XRING-pre) steps)


def build_nc(n_steps=S, unroll=8, prep_unroll=4, repeat=1, skip_prep=False,
             static_loops=False, pre_tiles=4, hint=False, stag=False,
             no_x4=False, no_cell=False, no_tp=False, no_mm=False):
    nc = bacc.Bacc("TRN2", target_bir_lowering=False, debug=False, num_devices=8)

    texts_d = nc.dram_tensor("texts", [S * B, 1], I32, kind="ExternalInput").ap()
    emb_d = nc.dram_tensor("emb", [V, E], F32, kind="ExternalInput").ap()
    wx0_d = nc.dram_tensor("wx0T", [E, G4], BF, kind="ExternalInput").ap()
    wh0_d = nc.dram_tensor("wh0T", [H, G4], BF, kind="ExternalInput").ap()
    wh1_d = nc.dram_tensor("wh1T", [H, G4], BF, kind="ExternalInput").ap()
    wy_d = nc.dram_tensor("wyT", [H, OUT], BF, kind="ExternalInput").ap()
    y_d = nc.dram_tensor("y", [B, OUT], F32, kind="ExternalOutput").ap()

    n_tiles = (n_steps * B) // 128
    pre = min(pre_tiles, n_tiles)

    with tile.TileContext(nc) as tc, ExitStack() as ctx:
        # ---------- constants / persistent weights ----------
        consts = ctx.enter_context(tc.tile_pool(name="consts", bufs=1))
        ident128 = consts.tile([128, 128], F32)
        make_identity(nc, ident128[:])
        ident64 = consts.tile([64, 64], F32)
        make_identity(nc, ident64[:])
        ident64b = consts.tile([128, 64], F32)  # identity at rows 64:128
        nc.gpsimd.memset(ident64b[:], 0.0)
        make_identity(nc, ident64b[64:128, :], nomemset=True)
        identx = consts.tile([128, 64], BF)  # [I64; 0] for the x4 psum-adds
        nc.gpsimd.memset(identx[:], 0.0)
        make_identity(nc, identx[0:64, :], nomemset=True)
        identxb = consts.tile([128, 64], BF)  # [0; I64]: odd-step token rows
        nc.gpsimd.memset(identxb[:], 0.0)
        make_identity(nc, identxb[64:128, :], nomemset=True)

        wpool = ctx.enter_context(tc.tile_pool(name="weights", bufs=1))
        wh0_sb = [wpool.tile([128, G4], BF, tag=f"wh0_{q}", name=f"wh0_{q}")
                  for q in range(NK)]
        wh1_sb = [wpool.tile([128, G4], BF, tag=f"wh1_{q}", name=f"wh1_{q}")
                  for q in range(NK)]
        wx0_sb = [wpool.tile([128, G4], BF, tag=f"wx0_{q}", name=f"wx0_{q}")
                  for q in range(NK)]
        wy_sb = wpool.tile([128, NK * OUT], BF)
        for q in range(NK):
            nc.sync.dma_start(wh0_sb[q][:], wh0_d[ts(q, 128), :])
            nc.sync.dma_start(wh1_sb[q][:], wh1_d[ts(q, 128), :])
            nc.sync.dma_start(wx0_sb[q][:], wx0_d[ts(q, 128), :])
            nc.sync.dma_start(wy_sb[:, ts(q, OUT)], wy_d[ts(q, 128), :])

        # ---------- state ----------
        state = ctx.enter_context(tc.tile_pool(name="state", bufs=1))
        h_sb = state.tile([128, NK * B], BF, tag="h")   # k-tile q at cols 64q
        h2_sb = state.tile([128, NK * B], BF, tag="h2")
        c_sb = state.tile([128, HH], F32, tag="c")      # hidden-folded
        c2_sb = state.tile([128, HH], F32, tag="c2")
        nc.vector.memset(h_sb[:].bitcast(F32), 0.0)
        nc.vector.memset(c_sb[:], 0.0)

        # ---------- pools ----------
        inner = ctx.enter_context(ExitStack())
        pbs = inner.enter_context(tc.tile_pool(name="pb_sb", bufs=3))
        pbx = inner.enter_context(tc.tile_pool(name="pb_xt", bufs=2))
        # x4 ring: consumed directly by the cells' identity-matmuls 2*pre
        # steps later.  The ring WAR is the real dependency that stops the
        # scheduler from running the whole prep phase ahead of the cells.
        pbo = inner.enter_context(tc.tile_pool(name="pb_out", bufs=XRING))
        # prep transposes and projection chunks share one [128,512] PSUM
        # rotation: 3 buffers of WAR slack so filler matmuls never wait on a
        # recent evacuation.
        pbp = inner.enter_context(tc.tile_pool(name="pb_ps", bufs=3, space="PSUM"))
        pcg = inner.enter_context(tc.tile_pool(name="pc_ps", bufs=2, space="PSUM"))
        pio = inner.enter_context(tc.tile_pool(name="pc_io", bufs=1, space="PSUM"))
        pct = inner.enter_context(tc.tile_pool(name="pc_tmp", bufs=2))
        pctp = inner.enter_context(tc.tile_pool(name="pc_tp", bufs=1, space="PSUM"))

        # ---------- phase B pieces: gather / transpose / project ----------
        # Split so the PE parts can be program-ordered into the recurrence's
        # chain-stall windows (between a cell's gate matmuls and its
        # h-transposes) — the PE queue is in-order, so filler must come
        # before the stalling instruction.
        def gather_piece(t):
            idx = pbs.tile([128, 1], I32, tag="idx")
            nc.sync.dma_start(idx[:], texts_d[ds(t * 128, 128), :])
            gx = pbs.tile([128, E], F32, tag="gx")
            nc.gpsimd.indirect_dma_start(
                out=gx[:],
                out_offset=None,
                in_=emb_d[:],
                in_offset=bass.IndirectOffsetOnAxis(ap=idx[:, :1], axis=0),
            )
            return gx

        def tp_mms(gx):
            # 4 PE transposes into one PSUM bank tile
            tp = pbp.tile([128, E], F32, tag="pps", name="pb_tp_t")
            for q in range(NK):
                nc.tensor.transpose(tp[:, ts(q, 128)], gx[:, ts(q, 128)],
                                    ident128[:])
            return tp

        def tp_evac(tp):
            xt = pbx.tile([128, E], BF, tag="xt", name="xt")
            nc.scalar.copy(xt[:], tp[:])
            return xt

        def tp_piece(gx):
            return tp_evac(tp_mms(gx))

        def chunk_mms(xt, n):
            ps = pbp.tile([128, 512], F32, tag="pps", name="x4ps")
            for q in range(NK):
                nc.tensor.matmul(
                    ps[:],
                    lhsT=xt[:, ts(q, 128)],
                    rhs=wx0_sb[q][:, ts(n, 512)],
                    start=(q == 0),
                    stop=(q == NK - 1),
                )
            return ps

        def chunk_evac(ps, x4o, n, eng):
            if eng == 0:
                nc.vector.tensor_copy(x4o[:, ts(n, 512)], ps[:])
            else:
                nc.scalar.copy(x4o[:, ts(n, 512)], ps[:])

        x4os = {}

        def prep_body(t):
            gx = gather_piece(t)
            xt = tp_piece(gx)
            x4o = pbo.tile([128, G4], BF, tag="x4o", name="x4o")
            x4os[t] = x4o
            for n in range(4):
                chunk_evac(chunk_mms(xt, n), x4o, n, n % 2)

        # ---------- recurrence cell ----------
        def cell(h_in, h_out, w_sb, c_in, c_out, x4sb, filler=None):
            FC = pcg.tile([128, 512], F32, tag="FC", name="FC")
            Ip = pio.tile([128, HH], F32, tag="Ip", name="Ip")
            Op = pio.tile([128, HH], F32, tag="Op", name="Op")

            def mmx(ps, col0, w):  # x4 pre-add into both halves of one group
                idsel, ring = x4sb
                nc.tensor.matmul(ps[0:64, :], lhsT=idsel[:],
                                 rhs=ring[:, ds(col0, w)],
                                 start=True, stop=False, tile_position=(0, 0),
                                 skip_group_check=True)
                nc.tensor.matmul(ps[64:128, :], lhsT=idsel[:],
                                 rhs=ring[:, ds(col0 + w, w)],
                                 start=True, stop=False, tile_position=(0, 64),
                                 skip_group_check=True)

            def mm(ps, col0, q, w):
                st = (q == 0 and x4sb is None)
                nc.tensor.matmul(ps[0:64, :], lhsT=h_in[:, ts(q, B)],
                                 rhs=w_sb[q][:, ds(col0, w)],
                                 start=st, stop=(q == NK - 1),
                                 tile_position=(0, 0), skip_group_check=True)
                nc.tensor.matmul(ps[64:128, :], lhsT=h_in[:, ts(q, B)],
                                 rhs=w_sb[q][:, ds(col0 + w, w)],
                                 start=st, stop=(q == NK - 1),
                                 tile_position=(0, 64), skip_group_check=True)

            if no_mm:
                hf0 = pct.tile([64, 2 * HH], F32, tag="hf", name="hf")
                nc.vector.tensor_copy(hf0[:, 0:256], c_in[0:64, :])
                nc.vector.tensor_copy(hf0[:, 256:512], c_in[64:128, :])
                nc.vector.tensor_copy(c_out[:], c_in[:])
                emit_tp(hf0, h_out)
                return

            # FC group, then I group, then O group: the sgI/t2/c' chain
            # fires after the I group and hides under the O-group matmuls.
            if x4sb is not None:
                mmx(FC, 0, 512)
                mmx(Ip, 1024, 256)
            for q in range(NK):
                mm(FC, 0, q, 512)
            sgF = pct.tile([128, HH], F32, tag="sgF", name="sgF")
            sgC = pct.tile([128, HH], F32, tag="sgC", name="sgC")
            nc.scalar.activation(sgF[:], FC[:, 0:256], AF.Sigmoid)
            nc.scalar.activation(sgC[:], FC[:, 256:512], AF.Tanh)
            t1 = pct.tile([128, HH], F32, tag="t1", name="t1")
            nc.vector.tensor_mul(t1[:], sgF[:], c_in[:])
            for q in range(NK):
                mm(Ip, 1024, q, 256)
            sgI = pct.tile([128, HH], F32, tag="sgI", name="sgI")
            nc.scalar.activation(sgI[:], Ip[:], AF.Sigmoid)
            t2 = pct.tile([128, HH], F32, tag="t2", name="t2")
            nc.vector.tensor_mul(t2[:], sgI[:], sgC[:])
            nc.vector.tensor_add(c_out[:], t1[:], t2[:])
            if x4sb is not None:
                mmx(Op, 1536, 256)
            for q in range(NK):
                mm(Op, 1536, q, 256)
            # PE filler: runs during the act/c-chain stall (must precede the
            # transposes in the in-order PE queue).  Returns a closure that
            # emits its ACT/DVE evacuations, deferred to the cell end so the
            # critical-chain ops stay ahead of them in those queues.
            post = filler() if filler is not None else None
            sgO = pct.tile([128, HH], F32, tag="sgO", name="sgO")
            nc.scalar.activation(sgO[:], Op[:], AF.Sigmoid)
            # tct and the h-multiplies in column halves so the k0 transpose
            # starts as early as possible; the bottom array-half shifts down
            # to partitions 0:64 so every transpose stays in PE row-group 0
            # (mixed row-groups into one PSUM bank fault on hardware).
            tct = pct.tile([128, HH], F32, tag="tct", name="tct")
            hf = pct.tile([64, 2 * HH], F32, tag="hf", name="hf")
            nc.scalar.activation(tct[:, 0:128], c_out[:, 0:128], AF.Tanh)
            nc.vector.tensor_mul(hf[:, 0:128], sgO[0:64, 0:128],
                                 tct[0:64, 0:128])
            nc.scalar.activation(tct[:, 128:256], c_out[:, 128:256], AF.Tanh)
            nc.vector.tensor_mul(hf[:, 128:256], sgO[0:64, 128:256],
                                 tct[0:64, 128:256])
            nc.vector.tensor_mul(hf[:, 256:384], sgO[64:128, 0:128],
                                 tct[64:128, 0:128])
            nc.vector.tensor_mul(hf[:, 384:512], sgO[64:128, 128:256],
                                 tct[64:128, 128:256])
            emit_tp(hf, h_out)
            if post is not None:
                post()

        def emit_tp(hf, h_out):
            if no_tp:
                nc.vector.tensor_copy(h_out[:], hf[0:128, 0:128])
                return
            # transpose back to k-tile lhsT layout; k0 first so the next
            # cell's first matmul unblocks as early as possible.
            tps = pctp.tile([128, NK * B], F32, name="tps")
            nc.tensor.transpose(tps[:, ts(0, B)], hf[:, 0:128], ident64[:])
            nc.scalar.copy(h_out[:, 0:64], tps[:, 0:64])
            nc.tensor.transpose(tps[:, ts(1, B)], hf[:, 128:256], ident64[:])
            nc.vector.tensor_copy(h_out[:, 64:128], tps[:, 64:128])
            nc.tensor.transpose(tps[:, ts(2, B)], hf[:, 256:384], ident64[:])
            nc.tensor.transpose(tps[:, ts(3, B)], hf[:, 384:512], ident64[:])
            nc.vector.tensor_copy(h_out[:, 128:256], tps[:, 128:256])

        def step_body(s, f0=None, f1=None):
            if no_cell:
                return
            if no_x4:
                cell(h_sb, h2_sb, wh0_sb, c_sb, c2_sb, None, f0)
            else:
                t = s // 2
                ring = x4os[t] if s % 2 == 0 else x4os.pop(t)
                idsel = identx if s % 2 == 0 else identxb
                cell(h_sb, h2_sb, wh0_sb, c_sb, c2_sb, (idsel, ring), f0)
            cell(h2_sb, h_sb, wh1_sb, c2_sb, c_sb, None, f1)

        # prep pipeline handles threaded across steps (build-time python)
        gxs, xts = {}, {}

        def stepprep_body(s):
            # transpose tile t_g this step; project tile t_g-1 (whose xt is
            # already a full step old, so the filler matmuls are never gated
            # on a same-step evacuation).
            t_g = s // 2 + pre
            tc_ = t_g - 1
            f0 = f1 = None
            if s % 2 == 0:
                if t_g < n_tiles:
                    if t_g not in gxs:  # bootstrap at s == 0
                        gxs[t_g] = gather_piece(t_g)
                    if t_g + 1 < n_tiles:
                        gxs[t_g + 1] = gather_piece(t_g + 1)

                    def f0():
                        tp = tp_mms(gxs.pop(t_g))
                        return lambda: xts.__setitem__(t_g, tp_evac(tp))

                if pre <= tc_ < n_tiles:
                    x4o = pbo.tile([128, G4], BF, tag="x4o", name="x4o")
                    x4os[tc_] = x4o

                    def f1():
                        ps0 = chunk_mms(xts[tc_], 0)
                        return lambda: chunk_evac(ps0, x4o, 0, 0)

                step_body(s, f0, f1)
            else:
                if pre <= tc_ < n_tiles:
                    xt = xts.pop(tc_)
                    x4o = x4os[tc_]

                    def f0():
                        ps1 = chunk_mms(xt, 1)
                        ps2 = chunk_mms(xt, 2)

                        def post():
                            chunk_evac(ps1, x4o, 1, 0)
                            chunk_evac(ps2, x4o, 2, 1)
                        return post

                    def f1():
                        ps3 = chunk_mms(xt, 3)
                        return lambda: chunk_evac(ps3, x4o, 3, 1)

                step_body(s, f0, f1)

        def stepprep_simple(s):
            step_body(s)
            if s % 2 == 0:
                prep_body(s // 2 + pre)

        # ---------- main loops ----------
        mid = min(2 * (n_tiles - pre), n_steps)
        if mid % 2:
            mid -= 1
        # +2: the shifted chunk pipeline finishes the last tile 2 steps late
        mid2 = min(mid + 2, n_steps) if mid > 0 else 0
        if skip_prep:
            loops = [(0, n_steps, step_body, unroll)]
        elif static_loops:
            loops = [(0, pre, prep_body, prep_unroll),
                     (0, mid2, stepprep_body, unroll),
                     (mid2, n_steps, step_body, unroll)]
        else:
            loops = [(0, pre, prep_body, prep_unroll),
                     (0, mid, stepprep_simple, unroll),
                     (mid, n_steps, step_body, unroll)]
        hint_engines = tuple(nc.engines.keys()) if hint else ()

        def run_loop(lo, hi, body, ur):
            n = hi - lo
            ur = min(ur, n)
            main = n - n % ur
            if main:
                with tc.For_i(lo, lo + main, ur, hint_engines=hint_engines,
                              staggered_reset=stag) as i:
                    for j in range(ur):
                        body(i + j)
            for r in range(lo + main, hi):
                body(r)

        for (lo, hi, body, ur) in loops:
            if hi <= lo:
                continue
            if static_loops:
                for i in range(lo, hi):
                    body(i)
            elif repeat > 1 and body is step_body:
                with tc.For_i(0, repeat, 1):
                    run_loop(lo, hi, body, ur)
            else:
                run_loop(lo, hi, body, ur)

        inner.close()

        # ---------- phase D: output projection ----------
        with tc.tile_pool(name="pd", bufs=1) as pd, \
             tc.tile_pool(name="pd_ps", bufs=1, space="PSUM") as pdp:
            yps = pdp.tile([64, OUT], F32)
            for q in range(NK):
                nc.tensor.matmul(
                    yps[:],
                    lhsT=h_sb[:, ts(q, B)],
                    rhs=wy_sb[:, ts(q, OUT)],
                    start=(q == 0),
                    stop=(q == NK - 1),
                )
            ysb = pd.tile([64, OUT], F32)
            nc.vector.tensor_copy(ysb[:], yps[:])
            nc.sync.dma_start(y_d[:], ysb[:])

    nc.compile()
    return nc


def prep_inputs(texts, emb, Wf, bf, Wi, bi, Wo, bo, Wc, bc, Wy, by):
    """Host-side layout prep. All heavy compute stays on device."""
    bf16 = mybir.dt.np(BF)
    texts = np.asarray(texts)
    sl, bb = texts.shape
    texts_i = np.ascontiguousarray(texts.reshape(sl * bb, 1).astype(np.int32))

    def fold_cols(wg):  # wg [2048(f,i,c,o), K] -> W^T [K, 2048] folded cols
        F, I, C, O = (wg[g * 512:(g + 1) * 512] for g in range(4))
        rows = np.concatenate([
            F[0:256], C[0:256], F[256:512], C[256:512],
            I[0:256], I[256:512], O[0:256], O[256:512]], axis=0)
        return np.ascontiguousarray(rows.T)

    w0 = np.concatenate([np.asarray(w)[0] for w in (Wf, Wi, Wc, Wo)], axis=0)
    w1 = np.concatenate([np.asarray(w)[1] for w in (Wf, Wi, Wc, Wo)], axis=0)
    wh0T = fold_cols(w0[:, :H])          # [512, 2048]
    wx0T = fold_cols(w0[:, H:])          # [512, 2048]
    wh1T = fold_cols(w1[:, :H] + w1[:, H:])
    wyT = np.ascontiguousarray(np.asarray(Wy).T)  # [512, 2]
    return {
        "texts": texts_i,
        "emb": np.ascontiguousarray(np.asarray(emb), dtype=np.float32),
        "wx0T": wx0T.astype(bf16),
        "wh0T": wh0T.astype(bf16),
        "wh1T": wh1T.astype(bf16),
        "wyT": wyT.astype(bf16),
    }


_NC_CACHE = {}


def kernel(**inputs) -> np.ndarray:
    key = "static"
    if key not in _NC_CACHE:
        _NC_CACHE[key] = build_nc(n_steps=S, static_loops=True)
    nc = _NC_CACHE[key]
    in_map = prep_inputs(**inputs)
    res = run_bass_kernel_spmd(nc, [in_map] * 8, core_ids=list(range(8)))
    return res.results[0]["y"]
